# revision 1
# baseline (speedup 1.0000x reference)
"""CTC decoder loss kernel for Trainium2 (8 NeuronCores, SPMD).

Strategy:
  - Data-parallel over batch: 16 samples -> 8 cores x 2 samples each.
  - Per core: PE GEMM (enc @ W, fp32) with fused exp+row-sum epilogue on ACT
    for logsumexp (no max subtraction: logits ~ N(0,1), exp is fp32-safe).
  - Gathered-vocab small GEMM (host gathers W columns for each sample's
    extended label sequence, two label positions packed per matmul) emits
    q = exp(ft*(glogit - lse)) directly in the recursion layout
    [partition = jhalf*64 + n*32 + chunk, t].
  - CTC alpha recursion in linear space: per step t, PE assembles
    y = shift1(alpha) + shift2(sigma) into PSUM via identity / subdiagonal
    matmuls (partition mixing covers chunk crossings), then DVE does
      w = y*ft_t + alpha ; alpha' = w*q_t ; sigma' = skip2*alpha'.
    sigma[s] stores skip(s+2)*alpha(s) so shift2(sigma) lands
    skip(s)*alpha(s-2).  Rescale by 1/sum every 8 steps against fp32
    underflow; the log of the scales accumulates on device.
  - Host gathers per-core outputs, reads alpha at the two end positions,
    adds back the accumulated log scale, reduces mean NLL.
"""
import os
import sys
import numpy as np

sys.path.insert(0, "/opt/trn_rl_repo")

# Problem constants (kernel.py is self-contained; shapes hardcoded).
N, T, D, V, L = 16, 512, 512, 4096, 128
S = 2 * L + 1          # 257 extended label positions
NCORES = 8
NL = N // NCORES       # 2 samples per core
C = 16                 # s-chunks per sample
J = 17                 # chunk width (C*J = 272 >= S)
PART = 48              # recursion partitions: n*32 + c, c in [0,16)
NPAIR = (J + 1) // 2   # 9 j-pairs for the small GEMM (last pair is single)
DC = D // 128          # 4 contraction chunks
VC = V // 512          # 8 vocab chunks
NT = NL * T            # 1024 GEMM rows per core
RESCALE_EVERY = 8

_CACHE = {}


def _legalize_waits(nc):
    """walrus in this container cannot encode >1 semaphore wait on one
    instruction: split extras onto single-wait NoOps inserted just before
    (same engine, in-order execution preserves semantics). Each NoOp bumps a
    fresh per-engine dummy semaphore (ids above anything the program uses) so
    the simulator's race tooling sees a real update; the dummies are never
    waited on.
    """
    import concourse.mybir as mybir
    max_id = 0
    for fn in nc.m.functions:
        for blk in fn.blocks:
            for inst in blk.instructions:
                si = inst.sync_info
                if si is None:
                    continue
                for w in (si.on_wait or []):
                    max_id = max(max_id, w.id)
                for u in (si.on_update or []):
                    max_id = max(max_id, u.id)
    dummies = {}

    def dummy_for(engine):
        if engine not in dummies:
            dummies[engine] = (max_id + 1 + len(dummies),
                               f"legal_dummy_{engine}")
        return dummies[engine]

    cnt = 0
    for fn in nc.m.functions:
        for blk in fn.blocks:
            new = []
            for inst in blk.instructions:
                si = inst.sync_info
                if si is not None and si.on_wait is not None and len(si.on_wait) > 1:
                    waits = list(si.on_wait)
                    for w in waits[:-1]:
                        cnt += 1
                        dmid, dmname = dummy_for(inst.engine)
                        new.append(mybir.InstNoOp(
                            name=f"legalw_{cnt}",
                            engine=inst.engine,
                            ins=[], outs=[],
                            sync_info=mybir.SyncInfo(
                                on_wait=[w],
                                on_update=[mybir.SyncUpdate(
                                    sync_type="semaphore", id=dmid,
                                    ant_name=dmname,
                                    update_mode="sem-inc", update_value=1)],
                            ),
                        ))
                    inst.sync_info = mybir.SyncInfo(
                        on_wait=[waits[-1]], on_update=list(si.on_update or []))
                new.append(inst)
            blk.instructions[:] = new
    return cnt


def _build_nc(t_steps):
    import concourse.bass as bass
    import concourse.mybir as mybir
    from concourse import tile

    fp32 = mybir.dt.float32
    AF = mybir.ActivationFunctionType
    ALU = mybir.AluOpType
    AX = mybir.AxisListType

    nc = bass.Bass()

    # ---- DRAM I/O (per core) ----
    encT_d = nc.dram_tensor("encT", [128, DC, NT], fp32, kind="ExternalInput")
    w_d = nc.dram_tensor("w", [128, DC, V], fp32, kind="ExternalInput")
    wg_d = nc.dram_tensor("wg", [128, NPAIR * NL * DC, 128], fp32, kind="ExternalInput")
    ft_d = nc.dram_tensor("ft", [PART, T], fp32, kind="ExternalInput")
    ftd_d = nc.dram_tensor("ftd", [128, T], fp32, kind="ExternalInput")
    skip2_d = nc.dram_tensor("skip2", [PART, J], fp32, kind="ExternalInput")
    e01_d = nc.dram_tensor("e01", [PART, J], fp32, kind="ExternalInput")
    ident_d = nc.dram_tensor("ident48", [PART, PART], fp32, kind="ExternalInput")
    shiftp_d = nc.dram_tensor("shiftp", [PART, PART], fp32, kind="ExternalInput")
    sumsel_d = nc.dram_tensor("sumsel", [PART, NL], fp32, kind="ExternalInput")
    sel2_d = nc.dram_tensor("sel2", [NL, PART], fp32, kind="ExternalInput")
    lsel_d = nc.dram_tensor("lsel", [NL, 128], fp32, kind="ExternalInput")
    padsel_d = nc.dram_tensor("padsel", [1, 128], fp32, kind="ExternalInput")

    alpha_out_d = nc.dram_tensor("alpha_out", [PART, J + 1], fp32, kind="ExternalOutput")
    logr_out_d = nc.dram_tensor("logr_out", [NL, 1], fp32, kind="ExternalOutput")

    with tile.TileContext(nc) as tc:
        with (
            tc.tile_pool(name="const", bufs=1) as const,
            tc.tile_pool(name="scratch", bufs=3) as scratch,
            tc.tile_pool(name="state", bufs=1) as state,
            tc.tile_pool(name="psA", bufs=2, space="PSUM") as psA,
            tc.tile_pool(name="psB", bufs=2, space="PSUM") as psB,
            tc.tile_pool(name="psY", bufs=3, space="PSUM") as psY,
            tc.tile_pool(name="psR", bufs=1, space="PSUM") as psR,
        ):
            # ---- constants / big persistent tiles ----
            encT = const.tile([128, DC, NT], fp32)
            nc.sync.dma_start(encT[:], encT_d[:])
            wfull = const.tile([128, DC, V], fp32)
            for dc in range(DC):
                for h in range(2):
                    nc.sync.dma_start(
                        wfull[:, dc, h * 2048:(h + 1) * 2048],
                        w_d[:, dc, h * 2048:(h + 1) * 2048],
                    )
            wg = const.tile([128, NPAIR * NL * DC, 128], fp32)
            nc.sync.dma_start(wg[:], wg_d[:])
            ft = const.tile([PART, T], fp32)
            nc.sync.dma_start(ft[:], ft_d[:])
            ftd = const.tile([128, T], fp32)
            nc.sync.dma_start(ftd[:], ftd_d[:])
            skip2 = const.tile([PART, J], fp32)
            nc.sync.dma_start(skip2[:], skip2_d[:])
            e01 = const.tile([PART, J], fp32)
            nc.sync.dma_start(e01[:], e01_d[:])
            ident48 = const.tile([PART, PART], fp32)
            nc.sync.dma_start(ident48[:], ident_d[:])
            shiftp = const.tile([PART, PART], fp32)
            nc.sync.dma_start(shiftp[:], shiftp_d[:])
            sumsel = const.tile([PART, NL], fp32)
            nc.sync.dma_start(sumsel[:], sumsel_d[:])
            sel2 = const.tile([NL, PART], fp32)
            nc.sync.dma_start(sel2[:], sel2_d[:])
            lsel = [const.tile([1, 128], fp32, tag=f"lsel{n}", name=f"lsel{n}")
                    for n in range(NL)]
            for n in range(NL):
                nc.sync.dma_start(lsel[n][:], lsel_d[n:n + 1, :])
            padsel = const.tile([1, 128], fp32)
            nc.sync.dma_start(padsel[:], padsel_d[:])

            lserow = [const.tile([1, T], fp32, tag=f"lserow{n}", name=f"lserow{n}") for n in range(NL)]
            ones_row = const.tile([1, T], fp32)
            nc.any.memset(ones_row[:], 1.0)

            # ---- phase 1: big GEMM + logsumexp ----
            for tt in range(NT // 128):
                n_idx = tt // (T // 128)
                t_off = (tt % (T // 128)) * 128
                sums = scratch.tile([128, VC], fp32, tag="sums")
                for vc in range(VC):
                    ps = psA.tile([128, 512], fp32, tag="gemm")
                    for dc in range(DC):
                        nc.tensor.matmul(
                            ps[:],
                            encT[:, dc, tt * 128:(tt + 1) * 128],
                            wfull[:, dc, vc * 512:(vc + 1) * 512],
                            start=(dc == 0),
                            stop=(dc == DC - 1),
                        )
                    dump = scratch.tile([128, 512], fp32, tag="dump")
                    nc.scalar.activation(
                        dump[:], ps[:], AF.Exp, accum_out=sums[:, vc:vc + 1]
                    )
                red = scratch.tile([128, 1], fp32, tag="red")
                nc.vector.tensor_reduce(red[:], sums[:], AX.X, ALU.add)
                lse_t = scratch.tile([128, 1], fp32, tag="lse_t")
                # lse' = Ln(sumexp/V): folds +lnV into q so q ~ O(1)/step
                nc.scalar.activation(lse_t[:], red[:], AF.Ln, scale=1.0 / 4096.0)
                nc.sync.dma_start(
                    lserow[n_idx][:, t_off:t_off + 128], lse_t[:]
                )

            # ---- phase 2: gathered-vocab GEMM -> qR ----
            # qR[p, j, t]: p = n*32 + c ; value q(t, s=17c+j, n)
            qR = const.tile([PART, J, T], fp32, tag="qR")
            for k in range(NPAIR):
                j0, j1 = 2 * k, 2 * k + 1
                psq = psB.tile([128, T], fp32, tag="psq")
                mm = 0
                for n in range(NL):
                    for dc in range(DC):
                        nc.tensor.matmul(
                            psq[:],
                            wg[:, (k * NL + n) * DC + dc, :],
                            encT[:, dc, n * T:(n + 1) * T],
                            start=(mm == 0), stop=False,
                        )
                        mm += 1
                for n in range(NL):
                    nc.tensor.matmul(
                        psq[:], lsel[n][:], lserow[n][:],
                        start=False, stop=(k == 0 and n == NL - 1),
                    )
                if k > 0:
                    nc.tensor.matmul(
                        psq[:], padsel[:], ones_row[:], start=False, stop=True,
                    )
                # q = exp(ft * (glog - lse)); frozen steps -> exp(0) = 1
                fq = scratch.tile([128, T], fp32, tag="fq")
                nc.vector.tensor_tensor(fq[:], psq[:], ftd[:], ALU.mult)
                nc.scalar.activation(qR[:, j0, :], fq[0:PART, :], AF.Exp)
                if j1 < J:
                    nc.scalar.activation(qR[:, j1, :], fq[64:64 + PART, :], AF.Exp)

            # ---- phase 3: recursion ----
            alpha_b = [state.tile([PART, 1 + J], fp32, tag=f"alpha{i}", name=f"alpha{i}") for i in range(2)]
            sigma_b = [state.tile([PART, 2 + J], fp32, tag=f"sigma{i}", name=f"sigma{i}") for i in range(2)]
            for i in range(2):
                nc.any.memset(alpha_b[i][:], 0.0)
                nc.any.memset(sigma_b[i][:], 0.0)
            logacc = state.tile([NL, 1], fp32)
            nc.any.memset(logacc[:], 0.0)

            nc.vector.tensor_tensor(
                alpha_b[0][:, 1:1 + J], qR[:, :, 0], e01[:], ALU.mult
            )
            nc.vector.tensor_tensor(
                sigma_b[0][:, 2:2 + J], alpha_b[0][:, 1:1 + J], skip2[:], ALU.mult
            )

            cur = 0
            for t in range(1, t_steps):
                al, sg = alpha_b[cur], sigma_b[cur]
                nal, nsg = alpha_b[1 - cur], sigma_b[1 - cur]
                y = psY.tile([PART, J], fp32, tag="y")
                nc.tensor.matmul(y[:], ident48[:], al[:, 0:J], start=True, stop=False)
                nc.tensor.matmul(
                    y[:, 0:1], shiftp[:], al[:, J:J + 1], start=False, stop=False,
                    skip_group_check=True,
                )
                nc.tensor.matmul(
                    y[:, 0:2], shiftp[:], sg[:, J:J + 2], start=False, stop=False,
                    skip_group_check=True,
                )
                nc.tensor.matmul(y[:], ident48[:], sg[:, 0:J], start=False, stop=True)
                w_t = scratch.tile([PART, J], fp32, tag="w_t")
                nc.vector.scalar_tensor_tensor(
                    w_t[:], y[:], ft[:, t:t + 1], al[:, 1:1 + J],
                    ALU.mult, ALU.add,
                )
                nc.vector.tensor_tensor(
                    nal[:, 1:1 + J], w_t[:], qR[:, :, t], ALU.mult
                )
                # sigma' on GPSIMD: off the DVE critical path (PE consumes
                # it next step; GPSIMD runs concurrently with DVE's i2)
                nc.gpsimd.tensor_tensor(
                    nsg[:, 2:2 + J], nal[:, 1:1 + J], skip2[:], ALU.mult
                )
                cur = 1 - cur

                if t % RESCALE_EVERY == RESCALE_EVERY - 1 or t == t_steps - 1:
                    al2, sg2 = alpha_b[cur], sigma_b[cur]
                    ps_r = psR.tile([NL, J], fp32, tag="rsc")
                    nc.tensor.matmul(
                        ps_r[:], sumsel[:], al2[:, 1:1 + J], start=True, stop=True
                    )
                    red_r = scratch.tile([NL, 1], fp32, tag="red_r")
                    nc.vector.tensor_reduce(red_r[:], ps_r[:], AX.X, ALU.add)
                    rinv = scratch.tile([NL, 1], fp32, tag="rinv")
                    nc.vector.reciprocal(rinv[:], red_r[:])
                    ps_e = psR.tile([PART, 1], fp32, tag="rsc")
                    nc.tensor.matmul(ps_e[:], sel2[:], rinv[:], start=True, stop=True)
                    scal = scratch.tile([PART, 1], fp32, tag="scal")
                    nc.vector.tensor_copy(scal[:], ps_e[:])
                    nc.vector.tensor_scalar_mul(
                        al2[:, 1:1 + J], al2[:, 1:1 + J], scal[:]
                    )
                    nc.vector.tensor_scalar_mul(
                        sg2[:, 2:2 + J], sg2[:, 2:2 + J], scal[:]
                    )
                    rs = scratch.tile([NL, 1], fp32, tag="rs")
                    nc.vector.tensor_scalar_mul(rs[:], red_r[:], float(2.0 ** -44))
                    lg = scratch.tile([NL, 1], fp32, tag="lg")
                    nc.scalar.activation(lg[:], rs[:], AF.Ln)
                    nc.vector.tensor_add(logacc[:], logacc[:], lg[:])

            nc.sync.dma_start(alpha_out_d[:], alpha_b[cur][:])
            nc.sync.dma_start(logr_out_d[:], logacc[:])

    _legalize_waits(nc)
    return nc


def _host_inputs(encoder_out, W, encoder_out_lens, padded_labels, label_lengths):
    enc = np.asarray(encoder_out, np.float32)
    W = np.asarray(W, np.float32)
    lens = np.asarray(encoder_out_lens)
    labels = np.asarray(padded_labels)
    llen = np.asarray(label_lengths)

    iden48 = np.eye(PART, dtype=np.float32)
    shiftp = np.zeros((PART, PART), np.float32)
    for m in range(PART):
        if m % 32 != 0 and m % 32 < C:
            shiftp[m - 1, m] = 1.0
    sumsel = np.zeros((PART, NL), np.float32)
    sel2 = np.zeros((NL, PART), np.float32)
    for n in range(NL):
        sumsel[n * 32:n * 32 + C, n] = 1.0
        sel2[n, n * 32:n * 32 + C] = 2.0 ** 64
    lsel = np.zeros((NL, 128), np.float32)
    for n in range(NL):
        for jh in range(2):
            lsel[n, jh * 64 + n * 32:jh * 64 + n * 32 + C] = -1.0
    padsel = np.zeros((1, 128), np.float32)
    for jh in range(2):
        for n in range(NL):
            padsel[0, jh * 64 + n * 32 + C - 1] = -1e9

    w_in = np.ascontiguousarray(W.reshape(DC, 128, V).transpose(1, 0, 2))

    in_maps, meta = [], []
    for core in range(NCORES):
        sl = slice(core * NL, (core + 1) * NL)
        enc_c, lens_c, labels_c, llen_c = enc[sl], lens[sl], labels[sl], llen[sl]

        # encT[di, dc, n*T+t] = enc[n, t, dc*128+di]
        encT = np.ascontiguousarray(
            enc_c.reshape(NL * T, DC, 128).transpose(2, 1, 0)
        )

        z = np.zeros((NL, S), np.int64)
        z[:, 1::2] = labels_c
        z_m2 = np.zeros_like(z)
        z_m2[:, 2:] = z[:, :-2]
        skip = (z != 0) & (z != z_m2)
        skip[:, :2] = False

        # wg[di, (k*NL+n)*DC+dc, m]; m = jh*64 + n*32 + c -> W[:, z[n, 17c+2k+jh]]
        wg = np.zeros((128, NPAIR * NL * DC, 128), np.float32)
        for k in range(NPAIR):
            for n in range(NL):
                for jh in range(2):
                    j = 2 * k + jh
                    if j >= J:
                        continue
                    for c in range(C):
                        s = c * J + j
                        if s < S:
                            col = W[:, z[n, s]].reshape(DC, 128)
                            for dc in range(DC):
                                wg[:, (k * NL + n) * DC + dc, jh * 64 + n * 32 + c] = col[dc]

        ft = np.zeros((PART, T), np.float32)
        skip2 = np.zeros((PART, J), np.float32)
        e01 = np.zeros((PART, J), np.float32)
        for n in range(NL):
            ftn = (np.arange(T)[None, :] < lens_c[n]).astype(np.float32)
            ft[n * 32:n * 32 + C, :] = ftn
            for c in range(C):
                for j in range(J):
                    s = c * J + j
                    if s + 2 < S:
                        skip2[n * 32 + c, j] = float(skip[n, s + 2])
            e01[n * 32, 0] = 1.0
            e01[n * 32, 1] = 1.0
        ftd = np.zeros((128, T), np.float32)
        ftd[0:PART] = ft
        ftd[64:64 + PART] = ft

        in_maps.append({
            "encT": encT, "w": w_in, "wg": wg, "ft": ft, "ftd": ftd,
            "skip2": skip2, "e01": e01, "ident48": iden48, "shiftp": shiftp,
            "sumsel": sumsel, "sel2": sel2, "lsel": lsel, "padsel": padsel,
        })
        meta.append((lens_c, llen_c))
    return in_maps, meta


def kernel(encoder_out, W, b, encoder_out_lens, padded_labels, label_lengths):
    from concourse.bass_utils import run_bass_kernel_spmd

    t_steps = int(os.environ.get("CTC_T_STEPS", T))
    if t_steps not in _CACHE:
        _CACHE[t_steps] = _build_nc(t_steps)
    nc = _CACHE[t_steps]

    in_maps, meta = _host_inputs(
        encoder_out, W, encoder_out_lens, padded_labels, label_lengths
    )
    res = run_bass_kernel_spmd(nc, in_maps, list(range(NCORES)))
    results = res.results

    bias = np.asarray(b, np.float64)
    assert np.allclose(bias, 0.0), "nonzero bias not supported"

    n_events = len([t for t in range(1, t_steps)
                    if t % RESCALE_EVERY == RESCALE_EVERY - 1 or t == t_steps - 1])
    ev_corr = n_events * 20.0 * np.log(2.0)

    nll = np.zeros(N, np.float64)
    for core in range(NCORES):
        alpha = np.asarray(results[core]["alpha_out"], np.float64)
        logr = np.asarray(results[core]["logr_out"], np.float64)
        lens_c, llen_c = meta[core]
        for n in range(NL):
            idx_blank = 2 * int(llen_c[n])
            tot = 0.0
            for s in (idx_blank, idx_blank - 1):
                c, j = divmod(s, J)
                tot += alpha[n * 32 + c, 1 + j]
            la = logr[n, 0] - ev_corr - min(int(lens_c[n]), t_steps) * np.log(4096.0)
            nll[core * NL + n] = -(np.log(tot) + la)
    return np.float32(np.sum(nll) / N)



# revision 3
# speedup vs baseline: 32.0247x; 32.0247x over previous
"""CTC decoder loss kernel for Trainium2 (8 NeuronCores, SPMD).

Strategy:
  - Data-parallel over batch: 16 samples -> 8 cores x 2 samples each.
  - Per core: PE GEMM (enc @ W, fp32) with fused exp+row-sum epilogue on ACT
    for logsumexp (no max subtraction: logits ~ N(0,1), exp is fp32-safe).
  - Gathered-vocab small GEMM (host gathers W columns for each sample's
    extended label sequence, two label positions packed per matmul) emits
    q = exp(ft*(glogit - lse)) directly in the recursion layout
    [partition = jhalf*64 + n*32 + chunk, t].
  - CTC alpha recursion in linear space: per step t, PE assembles
    y = shift1(alpha) + shift2(sigma) into PSUM via identity / subdiagonal
    matmuls (partition mixing covers chunk crossings), then DVE does
      w = y*ft_t + alpha ; alpha' = w*q_t ; sigma' = skip2*alpha'.
    sigma[s] stores skip(s+2)*alpha(s) so shift2(sigma) lands
    skip(s)*alpha(s-2).  Rescale by 1/sum every 8 steps against fp32
    underflow; the log of the scales accumulates on device.
  - Host gathers per-core outputs, reads alpha at the two end positions,
    adds back the accumulated log scale, reduces mean NLL.

Dispatch: the SPMD program is lowered through bass2jax's bass_exec
custom-call, but the jitted shard_map callable is built ONCE and cached —
run_bass_kernel_spmd rebuilds (and thus retraces/recompiles) it per call,
which costs seconds per dispatch.
"""
import os
import sys
import numpy as np

sys.path.insert(0, "/opt/trn_rl_repo")

# Problem constants (kernel.py is self-contained; shapes hardcoded).
N, T, D, V, L = 16, 512, 512, 4096, 128
S = 2 * L + 1          # 257 extended label positions
NCORES = 8
NL = N // NCORES       # 2 samples per core
C = 16                 # s-chunks per sample
J = 17                 # chunk width (C*J = 272 >= S)
PART = 48              # recursion partitions: n*32 + c, c in [0,16)
NPAIR = (J + 1) // 2   # 9 j-pairs for the small GEMM (last pair is single)
DC = D // 128          # 4 contraction chunks
VC = V // 512          # 8 vocab chunks
NT = NL * T            # 1024 GEMM rows per core
RESCALE_EVERY = 8

_CACHE = {}


def _legalize_waits(nc):
    """walrus in this container cannot encode >1 semaphore wait on one
    instruction: split extras onto single-wait NoOps inserted just before
    (same engine, in-order execution preserves semantics). Each NoOp bumps a
    fresh per-engine dummy semaphore (ids above anything the program uses) so
    the simulator's race tooling sees a real update; the dummies are never
    waited on.
    """
    import concourse.mybir as mybir
    max_id = 0
    for fn in nc.m.functions:
        for blk in fn.blocks:
            for inst in blk.instructions:
                si = inst.sync_info
                if si is None:
                    continue
                for w in (si.on_wait or []):
                    max_id = max(max_id, w.id)
                for u in (si.on_update or []):
                    max_id = max(max_id, u.id)
    dummies = {}

    def dummy_for(engine):
        if engine not in dummies:
            dummies[engine] = (max_id + 1 + len(dummies),
                               f"legal_dummy_{engine}")
        return dummies[engine]

    cnt = 0
    for fn in nc.m.functions:
        for blk in fn.blocks:
            new = []
            for inst in blk.instructions:
                si = inst.sync_info
                if si is not None and si.on_wait is not None and len(si.on_wait) > 1:
                    waits = list(si.on_wait)
                    for w in waits[:-1]:
                        cnt += 1
                        dmid, dmname = dummy_for(inst.engine)
                        new.append(mybir.InstNoOp(
                            name=f"legalw_{cnt}",
                            engine=inst.engine,
                            ins=[], outs=[],
                            sync_info=mybir.SyncInfo(
                                on_wait=[w],
                                on_update=[mybir.SyncUpdate(
                                    sync_type="semaphore", id=dmid,
                                    ant_name=dmname,
                                    update_mode="sem-inc", update_value=1)],
                            ),
                        ))
                    inst.sync_info = mybir.SyncInfo(
                        on_wait=[waits[-1]], on_update=list(si.on_update or []))
                new.append(inst)
            blk.instructions[:] = new
    return cnt


def _build_nc(t_steps):
    import concourse.bass as bass
    import concourse.mybir as mybir
    from concourse import tile

    fp32 = mybir.dt.float32
    AF = mybir.ActivationFunctionType
    ALU = mybir.AluOpType
    AX = mybir.AxisListType

    nc = bass.Bass()

    # ---- DRAM I/O (per core) ----
    encT_d = nc.dram_tensor("encT", [128, DC, NT], fp32, kind="ExternalInput")
    w_d = nc.dram_tensor("w", [128, DC, V], fp32, kind="ExternalInput")
    wg_d = nc.dram_tensor("wg", [128, NPAIR * NL * DC, 128], fp32, kind="ExternalInput")
    ft_d = nc.dram_tensor("ft", [PART, T], fp32, kind="ExternalInput")
    ftd_d = nc.dram_tensor("ftd", [128, T], fp32, kind="ExternalInput")
    skip2_d = nc.dram_tensor("skip2", [PART, J], fp32, kind="ExternalInput")
    e01_d = nc.dram_tensor("e01", [PART, J], fp32, kind="ExternalInput")
    ident_d = nc.dram_tensor("ident48", [PART, PART], fp32, kind="ExternalInput")
    shiftp_d = nc.dram_tensor("shiftp", [PART, PART], fp32, kind="ExternalInput")
    sumsel_d = nc.dram_tensor("sumsel", [PART, NL], fp32, kind="ExternalInput")
    sel2_d = nc.dram_tensor("sel2", [NL, PART], fp32, kind="ExternalInput")
    lsel_d = nc.dram_tensor("lsel", [NL, 128], fp32, kind="ExternalInput")
    padsel_d = nc.dram_tensor("padsel", [1, 128], fp32, kind="ExternalInput")

    alpha_out_d = nc.dram_tensor("alpha_out", [PART, J + 1], fp32, kind="ExternalOutput")
    logr_out_d = nc.dram_tensor("logr_out", [NL, 1], fp32, kind="ExternalOutput")

    with tile.TileContext(nc) as tc:
        with (
            tc.tile_pool(name="const", bufs=1) as const,
            tc.tile_pool(name="scratch", bufs=3) as scratch,
            tc.tile_pool(name="state", bufs=1) as state,
            tc.tile_pool(name="psA", bufs=2, space="PSUM") as psA,
            tc.tile_pool(name="psB", bufs=2, space="PSUM") as psB,
            tc.tile_pool(name="psY", bufs=3, space="PSUM") as psY,
            tc.tile_pool(name="psR", bufs=1, space="PSUM") as psR,
        ):
            # ---- constants / big persistent tiles ----
            encT = const.tile([128, DC, NT], fp32)
            nc.sync.dma_start(encT[:], encT_d[:])
            wfull = const.tile([128, DC, V], fp32)
            for dc in range(DC):
                for h in range(2):
                    nc.sync.dma_start(
                        wfull[:, dc, h * 2048:(h + 1) * 2048],
                        w_d[:, dc, h * 2048:(h + 1) * 2048],
                    )
            wg = const.tile([128, NPAIR * NL * DC, 128], fp32)
            nc.sync.dma_start(wg[:], wg_d[:])
            ft = const.tile([PART, T], fp32)
            nc.sync.dma_start(ft[:], ft_d[:])
            ftd = const.tile([128, T], fp32)
            nc.sync.dma_start(ftd[:], ftd_d[:])
            skip2 = const.tile([PART, J], fp32)
            nc.sync.dma_start(skip2[:], skip2_d[:])
            e01 = const.tile([PART, J], fp32)
            nc.sync.dma_start(e01[:], e01_d[:])
            ident48 = const.tile([PART, PART], fp32)
            nc.sync.dma_start(ident48[:], ident_d[:])
            shiftp = const.tile([PART, PART], fp32)
            nc.sync.dma_start(shiftp[:], shiftp_d[:])
            sumsel = const.tile([PART, NL], fp32)
            nc.sync.dma_start(sumsel[:], sumsel_d[:])
            sel2 = const.tile([NL, PART], fp32)
            nc.sync.dma_start(sel2[:], sel2_d[:])
            lsel = [const.tile([1, 128], fp32, tag=f"lsel{n}", name=f"lsel{n}")
                    for n in range(NL)]
            for n in range(NL):
                nc.sync.dma_start(lsel[n][:], lsel_d[n:n + 1, :])
            padsel = const.tile([1, 128], fp32)
            nc.sync.dma_start(padsel[:], padsel_d[:])

            lserow = [const.tile([1, T], fp32, tag=f"lserow{n}", name=f"lserow{n}") for n in range(NL)]
            ones_row = const.tile([1, T], fp32)
            nc.any.memset(ones_row[:], 1.0)

            # ---- phase 1: big GEMM + logsumexp ----
            for tt in range(NT // 128):
                n_idx = tt // (T // 128)
                t_off = (tt % (T // 128)) * 128
                sums = scratch.tile([128, VC], fp32, tag="sums")
                for vc in range(VC):
                    ps = psA.tile([128, 512], fp32, tag="gemm")
                    for dc in range(DC):
                        nc.tensor.matmul(
                            ps[:],
                            encT[:, dc, tt * 128:(tt + 1) * 128],
                            wfull[:, dc, vc * 512:(vc + 1) * 512],
                            start=(dc == 0),
                            stop=(dc == DC - 1),
                        )
                    dump = scratch.tile([128, 512], fp32, tag="dump")
                    nc.scalar.activation(
                        dump[:], ps[:], AF.Exp, accum_out=sums[:, vc:vc + 1]
                    )
                red = scratch.tile([128, 1], fp32, tag="red")
                nc.vector.tensor_reduce(red[:], sums[:], AX.X, ALU.add)
                lse_t = scratch.tile([128, 1], fp32, tag="lse_t")
                # lse' = Ln(sumexp/V): folds +lnV into q so q ~ O(1)/step
                nc.scalar.activation(lse_t[:], red[:], AF.Ln, scale=1.0 / 4096.0)
                nc.sync.dma_start(
                    lserow[n_idx][:, t_off:t_off + 128], lse_t[:]
                )

            # ---- phase 2: gathered-vocab GEMM -> qR ----
            # qR[p, j, t]: p = n*32 + c ; value q(t, s=17c+j, n)
            qR = const.tile([PART, J, T], fp32, tag="qR")
            for k in range(NPAIR):
                j0, j1 = 2 * k, 2 * k + 1
                psq = psB.tile([128, T], fp32, tag="psq")
                mm = 0
                for n in range(NL):
                    for dc in range(DC):
                        nc.tensor.matmul(
                            psq[:],
                            wg[:, (k * NL + n) * DC + dc, :],
                            encT[:, dc, n * T:(n + 1) * T],
                            start=(mm == 0), stop=False,
                        )
                        mm += 1
                for n in range(NL):
                    nc.tensor.matmul(
                        psq[:], lsel[n][:], lserow[n][:],
                        start=False, stop=(k == 0 and n == NL - 1),
                    )
                if k > 0:
                    nc.tensor.matmul(
                        psq[:], padsel[:], ones_row[:], start=False, stop=True,
                    )
                # q = exp(ft * (glog - lse)); frozen steps -> exp(0) = 1
                fq = scratch.tile([128, T], fp32, tag="fq")
                nc.vector.tensor_tensor(fq[:], psq[:], ftd[:], ALU.mult)
                nc.scalar.activation(qR[:, j0, :], fq[0:PART, :], AF.Exp)
                if j1 < J:
                    nc.scalar.activation(qR[:, j1, :], fq[64:64 + PART, :], AF.Exp)

            # ---- phase 3: recursion ----
            alpha_b = [state.tile([PART, 1 + J], fp32, tag=f"alpha{i}", name=f"alpha{i}") for i in range(2)]
            sigma_b = [state.tile([PART, 2 + J], fp32, tag=f"sigma{i}", name=f"sigma{i}") for i in range(2)]
            for i in range(2):
                nc.any.memset(alpha_b[i][:], 0.0)
                nc.any.memset(sigma_b[i][:], 0.0)
            logacc = state.tile([NL, 1], fp32)
            nc.any.memset(logacc[:], 0.0)

            nc.vector.tensor_tensor(
                alpha_b[0][:, 1:1 + J], qR[:, :, 0], e01[:], ALU.mult
            )
            nc.vector.tensor_tensor(
                sigma_b[0][:, 2:2 + J], alpha_b[0][:, 1:1 + J], skip2[:], ALU.mult
            )

            cur = 0
            for t in range(1, t_steps):
                al, sg = alpha_b[cur], sigma_b[cur]
                nal, nsg = alpha_b[1 - cur], sigma_b[1 - cur]
                y = psY.tile([PART, J], fp32, tag="y")
                nc.tensor.matmul(y[:], ident48[:], al[:, 0:J], start=True, stop=False)
                nc.tensor.matmul(
                    y[:, 0:1], shiftp[:], al[:, J:J + 1], start=False, stop=False,
                    skip_group_check=True,
                )
                nc.tensor.matmul(
                    y[:, 0:2], shiftp[:], sg[:, J:J + 2], start=False, stop=False,
                    skip_group_check=True,
                )
                nc.tensor.matmul(y[:], ident48[:], sg[:, 0:J], start=False, stop=True)
                w_t = scratch.tile([PART, J], fp32, tag="w_t")
                nc.vector.scalar_tensor_tensor(
                    w_t[:], y[:], ft[:, t:t + 1], al[:, 1:1 + J],
                    ALU.mult, ALU.add,
                )
                nc.vector.tensor_tensor(
                    nal[:, 1:1 + J], w_t[:], qR[:, :, t], ALU.mult
                )
                # sigma' on GPSIMD: off the DVE critical path (PE consumes
                # it next step; GPSIMD runs concurrently with DVE's i2)
                nc.gpsimd.tensor_tensor(
                    nsg[:, 2:2 + J], nal[:, 1:1 + J], skip2[:], ALU.mult
                )
                cur = 1 - cur

                if t % RESCALE_EVERY == RESCALE_EVERY - 1 or t == t_steps - 1:
                    al2, sg2 = alpha_b[cur], sigma_b[cur]
                    ps_r = psR.tile([NL, J], fp32, tag="rsc")
                    nc.tensor.matmul(
                        ps_r[:], sumsel[:], al2[:, 1:1 + J], start=True, stop=True
                    )
                    red_r = scratch.tile([NL, 1], fp32, tag="red_r")
                    nc.vector.tensor_reduce(red_r[:], ps_r[:], AX.X, ALU.add)
                    rinv = scratch.tile([NL, 1], fp32, tag="rinv")
                    nc.vector.reciprocal(rinv[:], red_r[:])
                    ps_e = psR.tile([PART, 1], fp32, tag="rsc")
                    nc.tensor.matmul(ps_e[:], sel2[:], rinv[:], start=True, stop=True)
                    scal = scratch.tile([PART, 1], fp32, tag="scal")
                    nc.vector.tensor_copy(scal[:], ps_e[:])
                    nc.vector.tensor_scalar_mul(
                        al2[:, 1:1 + J], al2[:, 1:1 + J], scal[:]
                    )
                    nc.vector.tensor_scalar_mul(
                        sg2[:, 2:2 + J], sg2[:, 2:2 + J], scal[:]
                    )
                    rs = scratch.tile([NL, 1], fp32, tag="rs")
                    nc.vector.tensor_scalar_mul(rs[:], red_r[:], float(2.0 ** -44))
                    lg = scratch.tile([NL, 1], fp32, tag="lg")
                    nc.scalar.activation(lg[:], rs[:], AF.Ln)
                    nc.vector.tensor_add(logacc[:], logacc[:], lg[:])

            nc.sync.dma_start(alpha_out_d[:], alpha_b[cur][:])
            nc.sync.dma_start(logr_out_d[:], logacc[:])

    _legalize_waits(nc)
    return nc


class _Dispatcher:
    """Jit-once SPMD dispatch of a Bass program over 8 cores.

    Mirrors concourse.bass2jax.run_bass_via_pjrt's multi-core path, but the
    shard_map-jitted callable is constructed a single time, so warm calls hit
    jax's C++ fast path instead of retracing + recompiling.
    """

    def __init__(self, nc):
        import jax
        import concourse.mybir as mybir
        from concourse import bass2jax
        from jax.experimental.shard_map import shard_map
        from jax.sharding import Mesh, PartitionSpec, NamedSharding

        bass2jax.install_neuronx_cc_hook()
        self._jax = jax
        self.nc = nc

        partition_name = (
            nc.partition_id_tensor.name if nc.partition_id_tensor else None
        )
        in_names, out_names, out_avals, zero_outs = [], [], [], []
        for alloc in nc.m.functions[0].allocations:
            if not isinstance(alloc, mybir.MemoryLocationSet):
                continue
            name = alloc.memorylocations[0].name
            if alloc.kind == "ExternalInput":
                if name != partition_name:
                    in_names.append(name)
            elif alloc.kind == "ExternalOutput":
                shape = tuple(alloc.tensor_shape)
                dtype = mybir.dt.np(alloc.dtype)
                out_names.append(name)
                out_avals.append(jax.core.ShapedArray(shape, dtype))
                zero_outs.append(np.zeros(shape, dtype))
        n_params = len(in_names)
        n_outs = len(out_avals)
        bind_in_names = in_names + out_names + (
            [partition_name] if partition_name else []
        )
        donate = tuple(range(n_params, n_params + n_outs))

        def _body(*args):
            operands = list(args)
            if partition_name is not None:
                operands.append(bass2jax.partition_id_tensor())
            outs = bass2jax._bass_exec_p.bind(
                *operands,
                out_avals=tuple(out_avals),
                in_names=tuple(bind_in_names),
                out_names=tuple(out_names),
                lowering_input_output_aliases=(),
                sim_require_finite=True,
                sim_require_nnan=True,
                nc=nc,
            )
            return tuple(outs)

        devices = jax.devices()[:NCORES]
        mesh = Mesh(np.asarray(devices), ("core",))
        self.sharded = jax.jit(
            shard_map(
                _body, mesh=mesh,
                in_specs=(PartitionSpec("core"),) * (n_params + n_outs),
                out_specs=(PartitionSpec("core"),) * n_outs,
                check_rep=False,
            ),
            donate_argnums=donate,
            keep_unused=True,
        )
        self.sharding = NamedSharding(mesh, PartitionSpec("core"))
        self.in_names = in_names
        self.out_names = out_names
        self.out_avals = out_avals
        self.zero_outs = zero_outs

    def fresh_zero_outs(self):
        return [
            np.zeros((NCORES * z.shape[0], *z.shape[1:]), z.dtype)
            for z in self.zero_outs
        ]

    def __call__(self, global_in: dict[str, np.ndarray]):
        """global_in: name -> core-stacked array [(NCORES*p0), ...]."""
        out_arrs = self.sharded(
            *[global_in[name] for name in self.in_names], *self.fresh_zero_outs()
        )
        return {
            name: np.asarray(out_arrs[i]).reshape(
                NCORES, *self.out_avals[i].shape
            )
            for i, name in enumerate(self.out_names)
        }


def _get_dispatcher(t_steps):
    if t_steps not in _CACHE:
        _CACHE[t_steps] = _Dispatcher(_build_nc(t_steps))
    return _CACHE[t_steps]


# Static index arrays for the wg gather: for each valid (k, jh, c) with
# j = 2k + jh < J and s = c*J + j < S, wg rows jh*64 + n*32 + c of matmul
# slot (k*NL + n)*DC + dc hold W[dc*128:(dc+1)*128, z[n, s]].
_k_g, _jh_g, _c_g, _s_g = [], [], [], []
for _k in range(NPAIR):
    for _jh in range(2):
        _j = 2 * _k + _jh
        if _j >= J:
            continue
        for _c in range(C):
            _s = _c * J + _j
            if _s < S:
                _k_g.append(_k)
                _jh_g.append(_jh)
                _c_g.append(_c)
                _s_g.append(_s)
_k_g = np.array(_k_g)
_jh_g = np.array(_jh_g)
_c_g = np.array(_c_g)
_s_g = np.array(_s_g)


def _host_inputs(encoder_out, W, encoder_out_lens, padded_labels, label_lengths):
    """Build the core-stacked (global) input arrays, vectorized."""
    enc = np.ascontiguousarray(np.asarray(encoder_out, np.float32))
    W = np.asarray(W, np.float32)
    lens = np.asarray(encoder_out_lens).reshape(NCORES, NL)
    labels = np.asarray(padded_labels).reshape(NCORES, NL, L)
    llen = np.asarray(label_lengths).reshape(NCORES, NL)

    # encT[g*128+di, dc, n*T+t] = enc[g*NL+n, t, dc*128+di]
    encT = np.ascontiguousarray(
        enc.reshape(NCORES, NL * T, DC, 128).transpose(0, 3, 2, 1)
    ).reshape(NCORES * 128, DC, NT)

    # w[g*128+di, dc, v] = W[dc*128+di, v]  (replicated per core)
    w_core = np.ascontiguousarray(W.reshape(DC, 128, V).transpose(1, 0, 2))
    w_in = np.broadcast_to(w_core, (NCORES, 128, DC, V)).reshape(
        NCORES * 128, DC, V
    )

    # Extended label sequence per sample: z[g, n, s]
    z = np.zeros((NCORES, NL, S), dtype=np.int64)
    z[:, :, 1::2] = labels
    z_m2 = np.zeros_like(z)
    z_m2[:, :, 2:] = z[:, :, :-2]
    skip = (z != 0) & (z != z_m2)
    skip[:, :, :2] = False

    # wg[g*128+di, (k*NL+n)*DC+dc, jh*64+n*32+c] = W[dc*128+di, z[g,n,s]]
    cols = W[:, z[:, :, _s_g]]               # [D, NCORES, NL, nval]
    cols = cols.reshape(DC, 128, NCORES, NL, len(_s_g))
    wg = np.zeros((NCORES, 128, NPAIR * NL * DC, 128), np.float32)
    for n in range(NL):
        m_idx = _jh_g * 64 + n * 32 + _c_g
        for dc in range(DC):
            wg[:, :, (_k_g * NL + n) * DC + dc, m_idx] = (
                cols[dc, :, :, n, :].transpose(1, 0, 2)
            )
    wg = wg.reshape(NCORES * 128, NPAIR * NL * DC, 128)

    # ft[g, n*32+c, t] = (t < lens[g, n]); rows c in [0, C)
    ftn = (np.arange(T)[None, None, :] < lens[:, :, None]).astype(np.float32)
    ft = np.zeros((NCORES, PART, T), np.float32)
    for n in range(NL):
        ft[:, n * 32:n * 32 + C, :] = ftn[:, n:n + 1, :]
    ftd = np.zeros((NCORES, 128, T), np.float32)
    ftd[:, 0:PART] = ft
    ftd[:, 64:64 + PART] = ft

    # skip2[g, n*32+c, j] = skip[g, n, c*J+j+2] (s+2 < S else 0)
    skip_pad = np.zeros((NCORES, NL, C * J + 2), np.float32)
    skip_pad[:, :, :S] = skip.astype(np.float32)
    sk = skip_pad[:, :, 2:2 + C * J].reshape(NCORES, NL, C, J)
    skip2 = np.zeros((NCORES, PART, J), np.float32)
    e01 = np.zeros((NCORES, PART, J), np.float32)
    for n in range(NL):
        skip2[:, n * 32:n * 32 + C, :] = sk[:, n]
        e01[:, n * 32, 0] = 1.0
        e01[:, n * 32, 1] = 1.0

    # Small per-core constants (identical across cores).
    iden48 = np.eye(PART, dtype=np.float32)
    shiftp = np.zeros((PART, PART), np.float32)
    for m in range(PART):
        if m % 32 != 0 and m % 32 < C:
            shiftp[m - 1, m] = 1.0
    sumsel = np.zeros((PART, NL), np.float32)
    sel2 = np.zeros((NL, PART), np.float32)
    for n in range(NL):
        sumsel[n * 32:n * 32 + C, n] = 1.0
        sel2[n, n * 32:n * 32 + C] = 2.0 ** 64
    lsel = np.zeros((NL, 128), np.float32)
    for n in range(NL):
        for jh in range(2):
            lsel[n, jh * 64 + n * 32:jh * 64 + n * 32 + C] = -1.0
    padsel = np.zeros((1, 128), np.float32)
    for jh in range(2):
        for n in range(NL):
            padsel[0, jh * 64 + n * 32 + C - 1] = -1e9

    def rep(a):
        return np.broadcast_to(a, (NCORES, *a.shape)).reshape(
            NCORES * a.shape[0], *a.shape[1:]
        )

    global_in = {
        "encT": encT,
        "w": w_in,
        "wg": wg,
        "ft": ft.reshape(NCORES * PART, T),
        "ftd": ftd.reshape(NCORES * 128, T),
        "skip2": skip2.reshape(NCORES * PART, J),
        "e01": e01.reshape(NCORES * PART, J),
        "ident48": rep(iden48),
        "shiftp": rep(shiftp),
        "sumsel": rep(sumsel),
        "sel2": rep(sel2),
        "lsel": rep(lsel),
        "padsel": rep(padsel),
    }
    meta = (lens, llen)
    return global_in, meta


def _postprocess(outs, meta, t_steps):
    lens, llen = meta
    alpha = np.asarray(outs["alpha_out"], np.float64)   # [NCORES, PART, J+1]
    logr = np.asarray(outs["logr_out"], np.float64)     # [NCORES, NL, 1]

    n_events = len([t for t in range(1, t_steps)
                    if t % RESCALE_EVERY == RESCALE_EVERY - 1 or t == t_steps - 1])
    ev_corr = n_events * 20.0 * np.log(2.0)

    nll = np.zeros(N, np.float64)
    for core in range(NCORES):
        for n in range(NL):
            idx_blank = 2 * int(llen[core, n])
            tot = 0.0
            for s in (idx_blank, idx_blank - 1):
                c, j = divmod(s, J)
                tot += alpha[core, n * 32 + c, 1 + j]
            la = (logr[core, n, 0] - ev_corr
                  - min(int(lens[core, n]), t_steps) * np.log(4096.0))
            nll[core * NL + n] = -(np.log(tot) + la)
    return np.float32(np.sum(nll) / N)


def kernel(encoder_out, W, b, encoder_out_lens, padded_labels, label_lengths):
    t_steps = int(os.environ.get("CTC_T_STEPS", T))
    disp = _get_dispatcher(t_steps)

    bias = np.asarray(b, np.float64)
    assert np.allclose(bias, 0.0), "nonzero bias not supported"

    global_in, meta = _host_inputs(
        encoder_out, W, encoder_out_lens, padded_labels, label_lengths
    )
    outs = disp(global_in)
    return _postprocess(outs, meta, t_steps)
